# revision 1
# baseline (speedup 1.0000x reference)
"""Trainium2 Bass kernel for IntraFrameNet (self-attention + conv head).

Math (per sample b):
  f = curr_features[b].reshape(C, M)                      # C=128, M=4096
  S = f^T f * C^-0.5   (symmetric, [M, M])
  P = softmax(S, axis=-1)
  feats1 = f @ P^T     ([C, M]);  x = [feats1; f]         # [2C, M]
  y = W1 @ x + b1 -> BN(inference) -> leaky_relu(0.01)
  pred = w2 @ y + b2                                      # [1, M]

Device strategy (data-parallel, 1 sample / core, 8 cores):
  - S^T chunks computed directly by PE: out[n_chunk, m_super] =
    matmul(lhsT=f[:, n_chunk], rhs=f[:, m_super]) -- no transposes of P needed.
  - exp on ScalarE (PSUM->SBUF, bf16 out) with free-dim accum. By symmetry of S
    the free-dim sum of exp(S^T[n, m]) over all m gives the softmax
    denominator D[n].
  - PV: out[c, m_super] += matmul(lhsT=fT[n_chunk], rhs=expS^T chunk) over all
    n chunks (fT = f^T, bf16, built once by PE transposes).
  - Softmax division is deferred: feats1 = O * Dinv_bcast, then
    y = W1a @ feats1 + W1b @ f (single PSUM accumulation; BN folded into
    W1a/W1b/bhead on the host), Prelu(0.01) with bias, then the w2 matmul.
  - The D/head pipeline for m-group g only needs chunks 8g..8g+7 of the last
    m-super, so it is emitted interleaved into the last super's chunk loop.
"""

import numpy as np

import concourse.bass as bass
from concourse import bacc
import concourse.mybir as mybir
import concourse.tile as tile
from concourse.bass_utils import run_bass_kernel_spmd
from concourse.masks import make_identity

B, C, H, W = 8, 128, 64, 64
M = H * W          # 4096
NCH = M // 128     # 32 chunks of n
SUP = 1024         # m columns per super-block
NSUP = M // SUP    # 4
CPS = SUP // 128   # 8 chunks per super
SCALE = float(C) ** -0.5
BN_EPS = 1e-5
LEAKY = 0.01

f32 = mybir.dt.float32
f32r = mybir.dt.float32r
bf16 = mybir.dt.bfloat16
AF = mybir.ActivationFunctionType


def _build():
    nc = bacc.Bacc("TRN2", target_bir_lowering=False)

    f_d = nc.dram_tensor("f", [C, M], f32r, kind="ExternalInput")
    w1aT_d = nc.dram_tensor("w1aT", [C, C], f32r, kind="ExternalInput")
    w1bT_d = nc.dram_tensor("w1bT", [C, C], f32r, kind="ExternalInput")
    bhead_d = nc.dram_tensor("bhead", [C, 1], f32, kind="ExternalInput")
    w2T_d = nc.dram_tensor("w2T", [C, 1], f32r, kind="ExternalInput")
    sel_d = nc.dram_tensor("sel", [CPS, CPS * 128], f32r, kind="ExternalInput")
    pred_d = nc.dram_tensor("pred", [1, M], f32, kind="ExternalOutput")

    with tile.TileContext(nc) as tc:
        with (
            tc.tile_pool(name="singles", bufs=1) as singles,
            tc.tile_pool(name="pbufp", bufs=3) as pbufp,
            tc.tile_pool(name="sbm", bufs=2) as sbm,
            tc.tile_pool(name="ps_s", bufs=2, space="PSUM") as ps_s,
            tc.tile_pool(name="ps_o", bufs=1, space="PSUM") as ps_o,
            tc.tile_pool(name="ps_h", bufs=2, space="PSUM") as ps_h,
            tc.tile_pool(name="dramp", bufs=1, space="DRAM") as dramp,
        ):
            # ---- load inputs (f split so compute can start early) ----
            fs = singles.tile([C, M], f32r)
            for q in range(2):
                nc.sync.dma_start(
                    out=fs[:, q * 512 : (q + 1) * 512],
                    in_=f_d[:, q * 512 : (q + 1) * 512],
                )
            w1aT = singles.tile([C, C], f32r)
            nc.sync.dma_start(out=w1aT, in_=w1aT_d[:, :])
            w1bT = singles.tile([C, C], f32r)
            nc.sync.dma_start(out=w1bT, in_=w1bT_d[:, :])
            bhead = singles.tile([C, 1], f32)
            nc.sync.dma_start(out=bhead, in_=bhead_d[:, :])
            w2T = singles.tile([C, 1], f32r)
            nc.sync.dma_start(out=w2T, in_=w2T_d[:, :])
            sel = singles.tile([CPS, CPS * 128], f32r)
            nc.sync.dma_start(out=sel, in_=sel_d[:, :])
            for q in range(2, 8):
                nc.sync.dma_start(
                    out=fs[:, q * 512 : (q + 1) * 512],
                    in_=f_d[:, q * 512 : (q + 1) * 512],
                )

            # ---- identities for PE transposes ----
            ident_bf = singles.tile([128, 128], bf16)
            make_identity(nc, ident_bf)
            ident_f32 = singles.tile([128, 128], f32)
            make_identity(nc, ident_f32)

            # ---- f in bf16 and fT (f transposed, bf16) ----
            fb = singles.tile([C, M], bf16)
            for q in range(8):
                nc.vector.tensor_copy(
                    out=fb[:, q * 512 : (q + 1) * 512],
                    in_=fs[:, q * 512 : (q + 1) * 512],
                )
            fT = singles.tile([128, NCH, 128], bf16)  # [n_local, chunk, c]

            def ft_prep(t):
                trp = ps_h.tile([128, 128], bf16, tag="ph", name=f"trp{t}")
                nc.tensor.transpose(trp, fb[:, t * 128 : (t + 1) * 128], ident_bf)
                nc.vector.tensor_copy(out=fT[:, t, :], in_=trp)

            # softmax denominator partials: pD[n_local, chunk, super]
            pD = singles.tile([128, NCH, NSUP], f32)
            # unnormalized feats1 (O = f @ expS^T), [c, m]
            O_sb = singles.tile([C, M], f32r)
            pred_sb = singles.tile([1, M], f32)

            drow_tiles = {}

            def head_d(g):
                """Softmax denominators for m-group g (DVE only)."""
                Dg = sbm.tile([128, CPS], f32, tag="Dg", name=f"Dg{g}")
                nc.vector.tensor_reduce(
                    out=Dg,
                    in_=pD[:, CPS * g : CPS * (g + 1), :],
                    axis=mybir.AxisListType.X,
                    op=mybir.AluOpType.add,
                )
                Dinvg = sbm.tile([128, CPS], f32, tag="Dinvg", name=f"Dinvg{g}")
                nc.vector.reciprocal(out=Dinvg, in_=Dg)
                drow_tiles[g] = Dinvg

            def head_t(g):
                """Transpose Dinv for m-group g (PE + DVE copy)."""
                Dinvg = drow_tiles[g]
                drpg = ps_h.tile([CPS, 128], f32, tag="ph", name=f"drpg{g}")
                nc.tensor.transpose(drpg, Dinvg, ident_f32)
                DrowTg = sbm.tile([CPS, 128], f32r, tag="DrowTg", name=f"DrowTg{g}")
                nc.vector.tensor_copy(out=DrowTg, in_=drpg)
                drow_tiles[g] = DrowTg

            fnorm_tiles = {}
            zsb_tiles = {}

            def head_pre(g):
                """Dinv broadcast + normalized feats1 (PE broadcast + DVE)."""
                DrowTg = drow_tiles[g]
                last = g == NSUP - 1
                pool, ptag = (ps_s, "st") if last else (ps_h, "ph")
                for h in range(2):
                    base = g * SUP + h * 512
                    hsl = bass.ds(base, 512)
                    dbp = pool.tile([128, 512], f32, tag=ptag, name=f"dbp{g}_{h}")
                    for j in range(4):
                        jj = h * 4 + j
                        nc.tensor.matmul(
                            dbp[:, j * 128 : (j + 1) * 128],
                            lhsT=sel[:, jj * 128 : (jj + 1) * 128],
                            rhs=DrowTg,
                            start=True,
                            stop=True,
                        )
                    dinvb = sbm.tile([128, 512], f32, tag="dinvb", name=f"dinvb{g}_{h}")
                    nc.vector.tensor_copy(out=dinvb, in_=dbp)
                    fnorm = sbm.tile([128, 512], f32r, tag="fnorm", name=f"fnorm{g}_{h}")
                    src0 = (
                        ot_tiles[g][:, h * 512 : (h + 1) * 512]
                        if last
                        else O_sb[:, hsl]
                    )
                    nc.vector.tensor_tensor(
                        out=fnorm, in0=src0, in1=dinvb, op=mybir.AluOpType.mult
                    )
                    fnorm_tiles[(g, h)] = fnorm

            def head_mid(g):
                """First conv (accumulated) + bias + leaky relu."""
                pool, ptag = (ps_s, "st") if g == NSUP - 1 else (ps_h, "ph")
                for h in range(2):
                    base = g * SUP + h * 512
                    hsl = bass.ds(base, 512)
                    yp = pool.tile([128, 512], f32, tag=ptag, name=f"yp{g}_{h}")
                    nc.tensor.matmul(
                        yp, lhsT=w1aT, rhs=fnorm_tiles.pop((g, h)), start=True,
                        stop=False,
                    )
                    nc.tensor.matmul(
                        yp, lhsT=w1bT, rhs=fs[:, hsl], start=False, stop=True
                    )
                    zsb = sbm.tile([128, 512], f32r, tag="zsb", name=f"zsb{g}_{h}")
                    if g == NSUP - 1:
                        # tail group: ACT is idle here, keep the fused Prelu
                        nc.scalar.activation(
                            out=zsb, in_=yp, func=AF.Prelu, bias=bhead, scale=1.0,
                            alpha=LEAKY,
                        )
                    else:
                        # mid-loop: ACT is the bottleneck -- leaky on DVE
                        t1 = sbm.tile([128, 512], f32, tag="t1", name=f"t1_{g}_{h}")
                        nc.vector.tensor_scalar_add(out=t1, in0=yp, scalar1=bhead)
                        nc.vector.scalar_tensor_tensor(
                            out=zsb, in0=t1, scalar=LEAKY, in1=t1,
                            op0=mybir.AluOpType.mult, op1=mybir.AluOpType.max,
                        )
                    zsb_tiles[(g, h)] = zsb

            def head_post(g):
                """Final 1-channel conv + pred copy."""
                pool, ptag = (ps_s, "st") if g == NSUP - 1 else (ps_h, "ph")
                for h in range(2):
                    base = g * SUP + h * 512
                    hsl = bass.ds(base, 512)
                    pp = pool.tile([1, 512], f32, tag=ptag, name=f"pp{g}_{h}")
                    nc.tensor.matmul(
                        pp, lhsT=w2T, rhs=zsb_tiles.pop((g, h)), start=True, stop=True
                    )
                    nc.vector.tensor_copy(out=pred_sb[0:1, hsl], in_=pp)

            # ---- main attention loop ----
            # Global chunk stream with 1-chunk S-matmul lookahead so an
            # eviction-stalled PV never blocks the next S (and hence exp).
            seq = [(s, t) for s in range(NSUP) for t in range(NCH)]
            st_tiles = {}
            ot_tiles = {}

            def emit_s(idx):
                s, t = seq[idx]
                st = ps_s.tile([128, SUP], f32, tag="st", name=f"st{s}_{t}")
                for q in range(2):
                    nc.tensor.matmul(
                        st[:, q * 512 : (q + 1) * 512],
                        lhsT=fs[:, t * 128 : (t + 1) * 128],
                        rhs=fs[:, s * SUP + q * 512 : s * SUP + (q + 1) * 512],
                        start=True,
                        stop=True,
                    )
                st_tiles[(s, t)] = st

            emit_s(0)
            for i, (s, t) in enumerate(seq):
                st = st_tiles.pop((s, t))
                pb = pbufp.tile([128, SUP], bf16, tag="pb", name=f"pb{s}_{t}")
                nc.scalar.activation(
                    out=pb,
                    in_=st,
                    func=AF.Exp,
                    scale=SCALE,
                    accum_out=pD[:, t, s : s + 1],
                )
                if i + 1 < len(seq):
                    emit_s(i + 1)
                if s == 0:
                    ft_prep(t)
                if t == 0:
                    ot_tiles[s] = ps_o.tile([C, SUP], f32, tag="ot", name=f"ot{s}")
                ot = ot_tiles[s]
                for q in range(2):
                    nc.tensor.matmul(
                        ot[:, q * 512 : (q + 1) * 512],
                        lhsT=fT[:, t, :],
                        rhs=pb[:, q * 512 : (q + 1) * 512],
                        start=(t == 0),
                        stop=(t == NCH - 1),
                    )
                if s == NSUP - 1:
                    gg, ph = t // CPS, t % CPS
                    if ph == CPS - 1 and gg < NSUP - 1:
                        head_d(gg)
                    elif ph == 1 and gg > 0:
                        head_t(gg - 1)
                    elif ph == 2 and gg > 0:
                        head_pre(gg - 1)
                    elif ph == 4 and gg > 0:
                        head_mid(gg - 1)
                    elif ph == 6 and gg > 0:
                        head_post(gg - 1)
                if t == NCH - 1 and s < NSUP - 1:
                    for q in range(2):
                        nc.vector.tensor_copy(
                            out=O_sb[:, s * SUP + q * 512 : s * SUP + (q + 1) * 512],
                            in_=ot[:, q * 512 : (q + 1) * 512],
                        )

            head_d(NSUP - 1)
            head_t(NSUP - 1)
            head_pre(NSUP - 1)
            head_mid(NSUP - 1)
            head_post(NSUP - 1)

            nc.sync.dma_start(out=pred_d[:, :], in_=pred_sb)

    nc.finalize()
    return nc


_NC = None


def _get_nc():
    global _NC
    if _NC is None:
        _NC = _build()
    return _NC


def kernel(**inputs):
    curr = np.asarray(inputs["curr_features"], np.float32)
    w1 = np.asarray(inputs["w1"], np.float32)
    b1 = np.asarray(inputs["b1"], np.float32)
    gamma = np.asarray(inputs["gamma"], np.float32)
    beta = np.asarray(inputs["beta"], np.float32)
    rm = np.asarray(inputs["running_mean"], np.float32)
    rv = np.asarray(inputs["running_var"], np.float32)
    w2 = np.asarray(inputs["w2"], np.float32)
    b2 = np.asarray(inputs["b2"], np.float32)

    # fold BN (inference) into the first conv
    a = gamma / np.sqrt(rv + BN_EPS)                      # [C]
    W1f = w1 * a[:, None]                                 # [C, 2C]
    bhead = (b1 * a + beta - rm * a).astype(np.float32).reshape(C, 1)
    w1aT = np.ascontiguousarray(W1f[:, :C].T, np.float32)  # feats1 part
    w1bT = np.ascontiguousarray(W1f[:, C:].T, np.float32)  # f part
    w2T = np.ascontiguousarray(w2.T, np.float32)           # [C, 1]

    selm = np.zeros((CPS, CPS * 128), np.float32)
    for k in range(CPS):
        selm[k, k * 128 : (k + 1) * 128] = 1.0

    nc = _get_nc()
    in_maps = []
    for b in range(B):
        in_maps.append(
            {
                "f": np.ascontiguousarray(curr[b].reshape(C, M)),
                "w1aT": w1aT,
                "w1bT": w1bT,
                "bhead": bhead,
                "w2T": w2T,
                "sel": selm,
            }
        )
    res = run_bass_kernel_spmd(nc, in_maps, core_ids=list(range(B)))
    preds = np.stack([r["pred"].reshape(1, H, W) for r in res.results], axis=0)
    return (preds + b2[0]).astype(np.float32)


if __name__ == "__main__":
    _build()
    print("build OK")

